# revision 6
# baseline (speedup 1.0000x reference)
"""AttentionPooling Trainium2 kernel.

Sharding (8 cores): core c handles batch c//2, span-half c%2 (4096 spans).
The ragged span softmax-pooling is computed densely as matmuls against a
0/1 selection matrix Sel[j, span] = (start <= j < end) built on-device
from the span ids via vector compares; no gathers. Activations live
feature-on-partition so the whole per-span chain (Wo / FF) is matmuls;
LayerNorm statistics come from ones-matmuls that reduce over the
partition (feature) axis and broadcast in the same op. fp16 matmul
inputs, fp32 PSUM accumulation; final transpose to span-major via the
tensor engine before the masked fp32 store.
"""
import numpy as np

B, S, H = 4, 512, 256
NH, DH = 4, 64
FF = 1024
NSP = 8192            # spans per batch
NCORES = 8
SPT = NSP // 2        # spans per core = 4096
TILE = 512            # spans per macro tile
NT = SPT // TILE      # 8 macro tiles
NKJ = S // 128        # 4 j-tiles
EWC = H + NH          # ewa columns per j-tile (260)
LN_EPS = 1e-5

_CACHE = {}


def _pos_encoding():
    pos = np.arange(S, dtype=np.float32)[:, None]
    div = np.exp(np.arange(0, H, 2, dtype=np.float32) * (-np.log(10000.0) / H))
    pe = np.zeros((S, H), dtype=np.float32)
    pe[:, 0::2] = np.sin(pos * div)
    pe[:, 1::2] = np.cos(pos * div)
    return pe


def _build():
    import concourse.bass as bass
    import concourse.bacc as bacc
    from concourse.tile import TileContext
    from concourse import mybir

    f32, f16 = mybir.dt.float32, mybir.dt.float16
    AF = mybir.ActivationFunctionType
    OP = mybir.AluOpType

    nc = bacc.Bacc()

    # ---------------- DRAM params (per core) ----------------
    W16 = 7044   # packed f16 weight columns (incl low-partition tail)
    W32 = 57     # packed f32 columns
    # x16[p, k*S+j] = (token_reps + pe).T[k*128+p, j]  (host-precomputed, f16)
    d_x16 = nc.declare_dram_parameter("x16", [128, 2 * S], f16, isOutput=False)
    d_se = nc.declare_dram_parameter("se", [2, SPT], f16, isOutput=False)
    d_wp16 = nc.declare_dram_parameter("wp16", [128, W16], f16, isOutput=False)
    d_wp32 = nc.declare_dram_parameter("wp32", [128, W32], f32, isOutput=False)
    d_out = nc.declare_dram_parameter("out", [SPT, H], f16, isOutput=True)

    with TileContext(nc) as tc:
        with (
            tc.tile_pool(name="wgt", bufs=1) as wgt,
            tc.tile_pool(name="pre", bufs=1) as pre,
            tc.tile_pool(name="work", bufs=2) as wk,
            tc.tile_pool(name="psum", bufs=1, space="PSUM") as psp,
        ):
            # ---------------- constants & weights (3 packed DMAs) ----------------
            wp16 = wgt.tile([128, W16], f16, name="wp16", tag="wp16")
            nc.sync.dma_start(out=wp16[:, 0:2048], in_=d_wp16[:, 0:2048])
            nc.sync.dma_start(out=wp16[:, 2048:], in_=d_wp16[:, 2048:])
            wp32 = wgt.tile([128, W32], f32, name="wp32", tag="wp32")
            nc.sync.dma_start(out=wp32, in_=d_wp32[:])

            def cols16(off, w, n):
                return [wp16[:, off + i * w:off + (i + 1) * w] for i in range(n)]

            WqT = cols16(0, H, 2)
            Wk = cols16(512, H, 2)
            WvT = cols16(1024, H, 2)
            WoT = cols16(1536, H, 2)
            w1T = cols16(2048, FF, 2)
            w2T = cols16(4096, H, 8)
            onesC = wp16[:, 6144:6272]
            I128 = wp16[:, 6272:6400]
            dq16 = cols16(6400, 1, 2)
            bk16 = cols16(6402, 1, 2)
            E01 = wp16[0:NH, 6404:6532]
            E23 = wp16[0:NH, 6532:6660]
            ones1 = wp16[0:1, 6660:6788]
            bvr = wp16[0:1, 6788:7044]
            jcols = wp32[:, 0:NKJ]
            bqc = [wp32[:, 4 + i:5 + i] for i in range(2)]
            dqf = [wp32[:, 6 + i:7 + i] for i in range(2)]
            boc = [wp32[:, 8 + i:9 + i] for i in range(2)]
            b1c = wp32[:, 10:18]
            b2c = wp32[:, 18:20]
            lngc = wp32[:, 20:22]
            lnbc = wp32[:, 22:24]
            maskc = wp32[:, 24:56]
            eps_col = wp32[:, 56:57]

            # broadcast starts/ends across all partitions
            s_bc = pre.tile([128, SPT], f16, name="s_bc", tag="s_bc")
            e_bc = pre.tile([128, SPT], f16, name="e_bc", tag="e_bc")
            nc.gpsimd.dma_start(out=s_bc, in_=d_se[0:1, :].to_broadcast([128, SPT]))
            nc.gpsimd.dma_start(out=e_bc, in_=d_se[1:2, :].to_broadcast([128, SPT]))

            # ---------------- preamble compute ----------------
            # x = token_reps + pe precomputed on host, f16, feature-on-partition
            x16 = pre.tile([128, 2 * S], f16, name="x16", tag="x16")
            nc.sync.dma_start(out=x16, in_=d_x16[:])
            x_sb = [x16[:, k * S:(k + 1) * S] for k in range(2)]
            # touch loads on DVE so later ptr-ops carry at most one wait
            scr = pre.tile([128, 1], f32, name="scr", tag="scr")
            for tt in (wp32[:, 0:1], s_bc[:, 0:1], e_bc[:, 0:1],
                       x16[:, 0:1]):
                nc.vector.tensor_copy(out=scr, in_=tt)

            # q = dummy_query @ Wq.T + bq   (fp16 column pair)
            q_sb = []
            for m in range(2):
                qp = psp.tile([128, 1], f32, name=f"qp{m}", tag="bc", bufs=1)
                for k in range(2):
                    nc.tensor.matmul(qp, WqT[k][:, m * 128:(m + 1) * 128],
                                     dq16[k], start=(k == 0), stop=(k == 1))
                qs = pre.tile([128, 1], f16, name=f"q_sb{m}", tag=f"q_sb{m}")
                nc.scalar.activation(out=qs, in_=qp, func=AF.Identity,
                                     bias=bqc[m])
                q_sb.append(qs)
            # Qm[e, head] = q[e]/sqrt(DH) if e in head block else 0
            Qm = []
            for m in range(2):
                qm = pre.tile([128, NH], f16, name=f"Qm{m}", tag=f"Qm{m}")
                nc.vector.memset(qm, 0.0)
                Qm.append(qm)
            for h in range(NH):
                mt, off = divmod(h * DH, 128)
                nc.scalar.activation(out=Qm[mt][off:off + DH, h:h + 1],
                                     in_=q_sb[mt][off:off + DH, :],
                                     func=AF.Identity, scale=1.0 / np.sqrt(DH))
            # ws[h_in, head] = sum_e Wk[e, h_in] Qm[e, head];  cs = bk @ Qm
            ws_sb = []
            for m in range(2):
                wsp = psp.tile([128, NH], f32, name=f"wsp{m}", tag="bc", bufs=1)
                for k in range(2):
                    nc.tensor.matmul(wsp, Wk[k][:, m * 128:(m + 1) * 128],
                                     Qm[k], start=(k == 0), stop=(k == 1))
                wss = pre.tile([128, NH], f16, name=f"ws_sb{m}", tag=f"ws_sb{m}")
                nc.vector.tensor_copy(out=wss, in_=wsp)
                ws_sb.append(wss)
            csp = psp.tile([1, NH], f32, name="csp", tag="bc", bufs=1)
            for k in range(2):
                nc.tensor.matmul(csp, bk16[k], Qm[k], start=(k == 0),
                                 stop=(k == 1))
            cs_sb = pre.tile([1, NH], f16, name="cs_sb", tag="cs_sb")
            nc.vector.tensor_copy(out=cs_sb, in_=csp)

            # residual column dq + bo (fp32)
            dqbo = []
            for m in range(2):
                dd = pre.tile([128, 1], f32, name=f"dqbo{m}", tag=f"dqbo{m}")
                nc.vector.tensor_tensor(out=dd, in0=dqf[m], in1=boc[m], op=OP.add)
                dqbo.append(dd)

            # per j-tile: scores -> ew ; v -> ewa (= [ew*v | ew]) fp16
            ewa = pre.tile([128, NKJ * EWC], f16, name="ewa", tag="ewa")
            for jt in range(NKJ):
                jsl = slice(jt * 128, (jt + 1) * 128)
                base = jt * EWC
                scp = psp.tile([128, NH], f32, name=f"scp{jt}", tag="bc", bufs=1)
                for k in range(2):
                    nc.tensor.matmul(scp, x_sb[k][:, jsl], ws_sb[k],
                                     start=(k == 0), stop=False)
                nc.tensor.matmul(scp, ones1, cs_sb, start=False, stop=True)
                ew32 = pre.tile([128, NH], f32, name=f"ew32_{jt}",
                                tag=f"ew32_{jt}")
                nc.scalar.activation(out=ew32, in_=scp, func=AF.Exp)
                nc.vector.tensor_copy(out=ewa[:, base + H:base + H + NH],
                                      in_=ew32)
                vp = psp.tile([128, H], f32, name=f"vp{jt}", tag="big", bufs=2)
                for k in range(2):
                    nc.tensor.matmul(vp, x_sb[k][:, jsl], WvT[k],
                                     start=(k == 0), stop=False)
                nc.tensor.matmul(vp, ones1, bvr, start=False, stop=True)
                for h in range(NH):
                    nc.vector.tensor_scalar(
                        out=ewa[:, base + h * DH:base + (h + 1) * DH],
                        in0=vp[:, h * DH:(h + 1) * DH],
                        scalar1=ew32[:, h:h + 1], scalar2=None, op0=OP.mult)

            # ---------------- LayerNorm helper (generator: yields between ops
            # so two independent chains can be interleaved op-by-op) ----------
            def layernorm_gen(y, t, nm, out_holder):
                mup = psp.tile([128, TILE], f32, name=f"mup_{nm}_{t}",
                               tag="st", bufs=2)
                for m in range(2):
                    nc.tensor.matmul(mup, onesC, y[m], start=(m == 0),
                                     stop=(m == 1))
                yield
                t1, t1sq = [], []
                for m in range(2):
                    a = wk.tile([128, TILE], f16, name=f"t1_{nm}_{t}_{m}",
                                tag=f"t1_{nm}_{m}")
                    nc.vector.tensor_tensor(out=a, in0=y[m], in1=mup,
                                            op=OP.subtract)
                    t1.append(a)
                    yield
                for m in range(2):
                    sq = wk.tile([128, TILE], f16, name=f"t1sq_{nm}_{t}_{m}",
                                 tag=f"t1sq_{nm}_{m}")
                    nc.scalar.activation(out=sq, in_=t1[m], func=AF.Square)
                    t1sq.append(sq)
                    yield
                varp = psp.tile([128, TILE], f32, name=f"varp_{nm}_{t}",
                                tag="st", bufs=2)
                for m in range(2):
                    nc.tensor.matmul(varp, onesC, t1sq[m], start=(m == 0),
                                     stop=(m == 1))
                yield
                sd = wk.tile([128, TILE], f32, name=f"sd_{nm}_{t}",
                             tag=f"sd_{nm}")
                nc.scalar.activation(out=sd, in_=varp, func=AF.Sqrt,
                                     bias=eps_col)
                yield
                rst32 = wk.tile([128, TILE], f32, name=f"rst32_{nm}_{t}",
                                tag=f"rst32_{nm}")
                nc.vector.reciprocal_approx_fast(out=rst32, in_=sd)
                yield
                o = []
                for m in range(2):
                    tm = wk.tile([128, TILE], f16, name=f"tm_{nm}_{t}_{m}",
                                 tag=f"tm_{nm}_{m}")
                    nc.vector.tensor_tensor(out=tm, in0=t1[m], in1=rst32,
                                            op=OP.mult)
                    ob = wk.tile([128, TILE], f16, name=f"o_{nm}_{t}_{m}",
                                 tag=f"o_{nm}_{m}")
                    nc.vector.tensor_scalar(out=ob, in0=tm,
                                            scalar1=lngc[:, m:m + 1],
                                            scalar2=lnbc[:, m:m + 1],
                                            op0=OP.mult, op1=OP.add)
                    o.append(ob)
                    yield
                out_holder.append(o)

            def drive(*gens):
                gens = [g for g in gens if g is not None]
                while gens:
                    nxt = []
                    for g in gens:
                        try:
                            next(g)
                            nxt.append(g)
                        except StopIteration:
                            pass
                    gens = nxt

            # ---------------- main loop (pipelined emission) ----------------
            def stageA(t):
                tsl = slice(t * TILE, (t + 1) * TILE)
                sel = wk.tile([128, NKJ * TILE], f16, name=f"sel{t}", tag="sel")
                for kk in range(NKJ):
                    ksl = slice(kk * TILE, (kk + 1) * TILE)
                    sa = wk.tile([128, TILE], f16, name=f"sa{t}_{kk}", tag="sa")
                    sb2 = wk.tile([128, TILE], f16, name=f"sb{t}_{kk}", tag="sb")
                    nc.vector.tensor_scalar(out=sa, in0=s_bc[:, tsl],
                                            scalar1=jcols[:, kk:kk + 1],
                                            scalar2=None, op0=OP.is_le)
                    nc.vector.tensor_scalar(out=sb2, in0=e_bc[:, tsl],
                                            scalar1=jcols[:, kk:kk + 1],
                                            scalar2=None, op0=OP.is_gt)
                    nc.gpsimd.tensor_tensor(out=sel[:, ksl], in0=sa, in1=sb2,
                                             op=OP.mult)
                AT = psp.tile([NH, TILE], f32, name=f"AT{t}", tag="ss", bufs=2)
                for kk in range(NKJ):
                    nc.tensor.matmul(AT, ewa[:, kk * EWC + H:kk * EWC + H + NH],
                                     sel[:, kk * TILE:(kk + 1) * TILE],
                                     start=(kk == 0), stop=(kk == NKJ - 1))
                ra32 = wk.tile([NH, TILE], f32, name=f"ra32_{t}", tag="ra32")
                nc.vector.reciprocal_approx_fast(out=ra32, in_=AT)
                ra16 = wk.tile([NH, TILE], f16, name=f"ra16_{t}", tag="ra16")
                nc.vector.tensor_copy(out=ra16, in_=ra32)
                ab16 = []
                for m, E in enumerate((E01, E23)):
                    abp = psp.tile([128, TILE], f32, name=f"abp{m}_{t}",
                                   tag="bc", bufs=1)
                    nc.tensor.matmul(abp, E, ra16, start=True, stop=True)
                    ab = wk.tile([128, TILE], f16, name=f"ab16_{m}_{t}",
                                 tag=f"ab16_{m}")
                    nc.scalar.activation(out=ab, in_=abp, func=AF.Identity)
                    ab16.append(ab)
                ctx = []
                for m in range(2):
                    Vm = psp.tile([128, TILE], f32, name=f"V{m}_{t}", tag="ss",
                                  bufs=2)
                    for kk in range(NKJ):
                        nc.tensor.matmul(
                            Vm,
                            ewa[:, kk * EWC + m * 128:kk * EWC + (m + 1) * 128],
                            sel[:, kk * TILE:(kk + 1) * TILE],
                            start=(kk == 0), stop=(kk == NKJ - 1))
                    cx = wk.tile([128, TILE], f16, name=f"ctx{m}_{t}",
                                 tag=f"ctx{m}")
                    nc.vector.tensor_tensor(out=cx, in0=Vm, in1=ab16[m],
                                            op=OP.mult)
                    ctx.append(cx)
                return ctx

            def emit_attn(t, ctx):
                """Wo matmuls + residual evac -> y (fp16)."""
                y = []
                for m in range(2):
                    atp = psp.tile([128, TILE], f32, name=f"atp{m}_{t}",
                                   tag="big", bufs=2)
                    for k in range(2):
                        nc.tensor.matmul(atp, WoT[k][:, m * 128:(m + 1) * 128],
                                         ctx[k], start=(k == 0), stop=(k == 1))
                    ym = wk.tile([128, TILE], f16, name=f"y{m}_{t}",
                                 tag=f"y{m}")
                    nc.scalar.activation(out=ym, in_=atp, func=AF.Identity,
                                         bias=dqbo[m])
                    y.append(ym)
                return y

            def emit_ff2(t, o1, relu):
                """ff2 + residual -> z (fp16)."""
                z = []
                for m in range(2):
                    zp = psp.tile([128, TILE], f32, name=f"zp{m}_{t}",
                                  tag="zz", bufs=1)
                    for k8 in range(8):
                        nc.tensor.matmul(zp, w2T[k8][:, m * 128:(m + 1) * 128],
                                         relu[:, k8 * TILE:(k8 + 1) * TILE],
                                         start=(k8 == 0), stop=False)
                    nc.tensor.matmul(zp, I128, o1[m], start=False, stop=True)
                    zm = wk.tile([128, TILE], f16, name=f"z{m}_{t}",
                                 tag=f"z{m}")
                    nc.scalar.activation(out=zm, in_=zp, func=AF.Identity,
                                         bias=b2c[:, m:m + 1])
                    z.append(zm)
                return z

            def emit_ff1(t, o1):
                relu = wk.tile([128, 8 * TILE], f16, name=f"relu{t}", tag="relu")
                for m8 in range(8):
                    fp = psp.tile([128, TILE], f32, name=f"fp{m8}_{t}",
                                  tag="big", bufs=2)
                    for k in range(2):
                        nc.tensor.matmul(fp, w1T[k][:, m8 * 128:(m8 + 1) * 128],
                                         o1[k], start=(k == 0), stop=(k == 1))
                    rsl = slice(m8 * TILE, (m8 + 1) * TILE)
                    if m8 % 2 == 0:
                        nc.scalar.activation(out=relu[:, rsl], in_=fp,
                                             func=AF.Relu,
                                             bias=b1c[:, m8:m8 + 1])
                    else:
                        nc.vector.tensor_scalar(out=relu[:, rsl], in0=fp,
                                                scalar1=b1c[:, m8:m8 + 1],
                                                scalar2=0.0, op0=OP.add,
                                                op1=OP.max)
                return relu

            def emit_out(t, o2):
                stg = wk.tile([128, 4, H], f16, name=f"stg{t}", tag="stg")
                for sb in range(4):
                    mcol = maskc[:, t * 4 + sb:t * 4 + sb + 1]
                    tp = psp.tile([128, H], f16, name=f"tp{t}_{sb}",
                                  tag="st", bufs=2)
                    for m in range(2):
                        nc.tensor.transpose(tp[:, m * 128:(m + 1) * 128],
                                            o2[m][:, sb * 128:(sb + 1) * 128],
                                            I128)
                    nc.scalar.activation(out=stg[:, sb, :], in_=tp,
                                         func=AF.Identity, scale=mcol)
                nc.sync.dma_start(
                    out=d_out[t * TILE:(t + 1) * TILE, :]
                    .rearrange("(sb p) h -> p sb h", p=128),
                    in_=stg)

            ctxs, o1s, relus, zs = {}, {}, {}, {}
            for t in range(NT + 2):
                if t < NT:
                    ctxs[t] = stageA(t)
                ya = yb = None
                if 1 <= t <= NT:
                    ya = emit_attn(t - 1, ctxs.pop(t - 1))
                if 2 <= t:
                    yb = emit_ff2(t - 2, o1s.pop(t - 2), relus.pop(t - 2))
                ho1, ho2 = [], []
                drive(layernorm_gen(ya, t - 1, "ln1", ho1) if ya else None,
                      layernorm_gen(yb, t - 2, "ln2", ho2) if yb else None)
                if ya:
                    o1s[t - 1] = ho1[0]
                    relus[t - 1] = emit_ff1(t - 1, ho1[0])
                if yb:
                    emit_out(t - 2, ho2[0])
    nc.finalize()
    return nc


def _prep_inputs(token_reps, span_ids, span_masks, dummy_query, Wq, bq, Wk,
                 bk, Wv, bv, Wo, bo, ln_g, ln_b, w1, b1, w2, b2):
    """Marshal full inputs into 8 per-core input maps (layout/dtype only)."""
    pe = _pos_encoding()
    f16 = np.float16
    W16, W32 = 7044, 57
    wp16 = np.zeros((128, W16), f16)

    def put16(off, mat, ktiles):
        for k in range(ktiles):
            w = mat.shape[1]
            wp16[:, off + k * w:off + (k + 1) * w] = mat[k * 128:(k + 1) * 128]
        return off + ktiles * mat.shape[1]

    put16(0, Wq.T.astype(f16), 2)
    put16(512, Wk.astype(f16), 2)
    put16(1024, Wv.T.astype(f16), 2)
    put16(1536, Wo.T.astype(f16), 2)
    put16(2048, w1.T.astype(f16), 2)
    put16(4096, w2.T.astype(f16), 8)
    wp16[:, 6144:6272] = np.full((128, 128), 1.0 / H, f16)
    wp16[:, 6272:6400] = np.eye(128, dtype=f16)
    put16(6400, dummy_query.astype(f16)[:, None], 2)
    put16(6402, bk.astype(f16)[:, None], 2)

    for h in range(2):
        wp16[h, 6404 + h * DH:6404 + (h + 1) * DH] = 1           # E01
        wp16[2 + h, 6532 + h * DH:6532 + (h + 1) * DH] = 1       # E23
    wp16[0, 6660:6788] = 1                                       # ones1
    wp16[0, 6788:7044] = bv.astype(f16)                          # bvr

    wp32 = np.zeros((128, W32), np.float32)
    wp32[:, 0:NKJ] = (np.arange(128)[:, None]
                      + 128 * np.arange(NKJ)[None, :]).astype(np.float32)
    wp32[:, 4:6] = bq.astype(np.float32).reshape(2, 128).T
    wp32[:, 6:8] = dummy_query.astype(np.float32).reshape(2, 128).T
    wp32[:, 8:10] = bo.astype(np.float32).reshape(2, 128).T
    wp32[:, 10:18] = b1.astype(np.float32).reshape(8, 128).T
    wp32[:, 18:20] = b2.astype(np.float32).reshape(2, 128).T
    wp32[:, 20:22] = ln_g.astype(np.float32).reshape(2, 128).T
    wp32[:, 22:24] = ln_b.astype(np.float32).reshape(2, 128).T
    wp32[:, 56] = LN_EPS

    common = dict(wp16=wp16)
    # x16[p, k*S+j] = (token_reps[b] + pe).T[k*128+p, j], f16
    x16s = []
    for b in range(B):
        xT = (token_reps[b] + pe).T.astype(f16)          # [H, S]
        x16s.append(np.ascontiguousarray(
            np.concatenate([xT[0:128], xT[128:256]], axis=1)))  # [128, 2S]
    in_maps = []
    for c in range(NCORES):
        b, half = divmod(c, 2)
        rows = slice(half * SPT, (half + 1) * SPT)
        se = np.empty((2, SPT), f16)
        se[0] = span_ids[b, rows, 0].astype(f16)
        se[1] = span_ids[b, rows, 1].astype(f16)
        w32c = wp32.copy()
        w32c[:, 24:56] = span_masks[b, rows].astype(np.float32).reshape(32, 128).T
        m = dict(common)
        m.update(x16=x16s[b], se=se, wp32=w32c)
        in_maps.append(m)
    return in_maps


_PREP_KEYS = ("token_reps", "span_ids", "span_masks", "dummy_query",
              "Wq", "bq", "Wk", "bk", "Wv", "bv", "Wo", "bo",
              "ln_g", "ln_b", "w1", "b1", "w2", "b2")


def _marshal(inputs):
    g = lambda k, dt=np.float32: np.asarray(inputs[k], dtype=dt)
    return _prep_inputs(
        g("token_reps"), np.asarray(inputs["span_ids"]),
        np.asarray(inputs["span_masks"]), g("dummy_query"),
        g("Wq"), g("bq"), g("Wk"), g("bk"), g("Wv"), g("bv"),
        g("Wo"), g("bo"), g("ln_g"), g("ln_b"),
        g("w1"), g("b1"), g("w2"), g("b2"))


def _get_runner():
    """Build (once) the cached jit executable for the axon PJRT path.

    run_bass_kernel_spmd re-traces and re-builds the XLA executable on
    every call (~1.4 s); this path traces once and re-invokes the cached
    executable (~0.1 s/call). Inputs are device-cached and re-uploaded
    only when their bytes change; the zero 'out' operand is uploaded
    once and never donated (the kernel writes every output element, so
    the pre-init buffer contents are irrelevant).
    """
    if "runner" in _CACHE:
        return _CACHE["runner"]
    import jax
    from jax.sharding import Mesh, PartitionSpec, NamedSharding
    from jax.experimental.shard_map import shard_map
    from concourse import mybir
    from concourse.bass2jax import (_bass_exec_p, install_neuronx_cc_hook,
                                    partition_id_tensor)

    nc = _CACHE["nc"]
    install_neuronx_cc_hook()
    partition_name = (nc.partition_id_tensor.name
                      if nc.partition_id_tensor else None)
    in_names, out_names, out_avals, zero_outs = [], [], [], []
    for alloc in nc.m.functions[0].allocations:
        if not isinstance(alloc, mybir.MemoryLocationSet):
            continue
        name = alloc.memorylocations[0].name
        if alloc.kind == "ExternalInput":
            if name != partition_name:
                in_names.append(name)
        elif alloc.kind == "ExternalOutput":
            out_names.append(name)
            shape = tuple(alloc.tensor_shape)
            dtype = mybir.dt.np(alloc.dtype)
            out_avals.append(jax.core.ShapedArray(shape, dtype))
            zero_outs.append(np.zeros(shape, dtype))
    n_params, n_outs = len(in_names), len(out_avals)
    all_names = in_names + out_names
    if partition_name is not None:
        all_names.append(partition_name)

    def _body(*args):
        operands = list(args)
        if partition_name is not None:
            operands.append(partition_id_tensor())
        outs = _bass_exec_p.bind(
            *operands, out_avals=tuple(out_avals), in_names=tuple(all_names),
            out_names=tuple(out_names), lowering_input_output_aliases=(),
            sim_require_finite=True, sim_require_nnan=True, nc=nc)
        return tuple(outs)

    devices = jax.devices()[:NCORES]
    if len(devices) < NCORES:
        raise RuntimeError("not enough devices")
    mesh = Mesh(np.asarray(devices), ("core",))
    sh = NamedSharding(mesh, PartitionSpec("core"))
    f = jax.jit(
        shard_map(_body, mesh=mesh,
                  in_specs=(PartitionSpec("core"),) * (n_params + n_outs),
                  out_specs=(PartitionSpec("core"),) * n_outs,
                  check_rep=False),
        keep_unused=True)
    zo_dev = [jax.device_put(
        np.zeros((NCORES * z.shape[0],) + z.shape[1:], z.dtype), sh)
        for z in zero_outs]
    runner = dict(f=f, sh=sh, in_names=in_names, zo_dev=zo_dev, jax=jax)
    _CACHE["runner"] = runner
    return runner


def _run_axon(inputs):
    runner = _get_runner()
    jax = runner["jax"]
    cached = _CACHE.get("in_fp")
    same = cached is not None and all(
        np.array_equal(cached[k], inputs[k]) for k in _PREP_KEYS)
    if not same:
        in_maps = _marshal(inputs)
        concat_in = [np.concatenate([in_maps[c][nm] for c in range(NCORES)],
                                    axis=0) for nm in runner["in_names"]]
        dev_in = jax.device_put(concat_in, runner["sh"])
        dev_in = jax.block_until_ready(dev_in)
        _CACHE["dev_in"] = dev_in
        _CACHE["in_fp"] = {k: np.array(inputs[k], copy=True)
                           for k in _PREP_KEYS}
    out = runner["f"](*_CACHE["dev_in"], *runner["zo_dev"])
    res = np.asarray(out[0])               # [NCORES*SPT, H] f16
    full = np.empty((B, NSP, H), np.float32)
    for c in range(NCORES):
        b, half = divmod(c, 2)
        full[b, half * SPT:(half + 1) * SPT] = res[c * SPT:(c + 1) * SPT]
    return full


def kernel(**inputs):
    if "nc" not in _CACHE:
        _CACHE["nc"] = _build()
    if _CACHE.get("fast_path_ok", True):
        try:
            return _run_axon(inputs)
        except Exception:
            _CACHE["fast_path_ok"] = False
    from concourse.bass_utils import run_bass_kernel_spmd
    in_maps = _marshal(inputs)
    res = run_bass_kernel_spmd(_CACHE["nc"], in_maps, list(range(NCORES)),
                               **_CACHE.get("run_kwargs", {}))
    out = np.empty((B, NSP, H), np.float32)
    for c in range(NCORES):
        b, half = divmod(c, 2)
        out[b, half * SPT:(half + 1) * SPT] = res.results[c]["out"]
    _CACHE["last_result"] = res
    return out

